# revision 45
# baseline (speedup 1.0000x reference)
"""Trainium2 Bass kernel for nn_CausalLayer (bilinear causal mixing layer).

Math (per batch b):
    E = ae[x]                                # [L, D] gather
    S[i,j] = E_i @ w @ E_j                   # bilinear pairwise score
    coef[i,j] = (i+1)/(j+1) for i<j else 0
    res[:,j] = bx[:,j] + sum_i coef[i,j]*S[i,j]*bx[:,i]

Chunked linear-attention identity, per 128-token chunk c with
a'_i = (i+1) * (w^T e_i):
    res_j = bx_j + (1/(j+1)) * [ E_j @ M_c + sum_{i<j in c} (a'_i . e_j) bx_i ]
    M_c   = sum_{i in chunks < c} a'_i bx_i^T      ([D, H] running state)

Host prep: the fused gather table row [ae[v] | (ae@w)[v]] is gathered and
position-scaled on host, and handed to the device in the two layouts the PE
consumes: A' token-major ([ROWS, D], the Mup stationary operand) and
[Et | A't] d-major per chunk ([NCH*D, 4C], S/EM stationary operands, both
batches side by side). This removes all on-device transposes/copies for the
score path at the cost of ~1MB/core of extra DMA.

Device: the two local batches are interleaved per chunk-step. All PE
operands sit at SBUF partition base 0 (mixed-base row-tiled matmul pairs
hang TRN2); per-batch data is side-by-side on the free axis, with the M
state as one [64, 2H] PSUM accumulator (bank-aligned splits). Wire dtypes
bf16 (incl. the output, upcast on host); f32 accumulation in PSUM; the mask
is a constant 0/1 strictly-upper [128,128] tile.

Sharding: batch-parallel, 2 of 16 batches per core, no cross-core comms.
"""

import os
import sys

for _p in ("/opt/trn_rl_repo", "/root/.axon_site/_ro/trn_rl_repo"):
    if os.path.isdir(_p) and _p not in sys.path:
        sys.path.insert(0, _p)

import numpy as np

B, L, H = 16, 2048, 768
V, D = 30000, 64
NCORES = 8
BPC = B // NCORES          # batches per core
C = 128                    # chunk (tile) size along sequence
NCH = L // C               # chunks per batch
ROWS = BPC * L             # rows per core
NP2 = NCH // 2             # chunk pairs per batch

_compiled = {}

# PSUM-bank-aligned column splits for the [64, 2H] M accumulator
MUP_SPLIT = (((0, 512), (512, 768)), ((0, 256), (256, 768)))


def _build():
    key = ("v6", os.environ.get("KWARM", "9"))
    if key in _compiled:
        return _compiled[key]

    import concourse.bacc as bacc
    import concourse.bass as bass
    import concourse.mybir as mybir
    import concourse.tile as tile

    f32 = mybir.dt.float32
    bf16 = mybir.dt.bfloat16
    mult = mybir.AluOpType.mult
    add = mybir.AluOpType.add

    nc = bacc.Bacc(
        "TRN2",
        target_bir_lowering=False,
        debug=False,
        enable_asserts=False,
        num_devices=NCORES,
    )

    bx_d = nc.dram_tensor("bx", [ROWS, H], bf16, kind="ExternalInput").ap()
    ap_d = nc.dram_tensor("apm", [ROWS, D], bf16, kind="ExternalInput").ap()
    eat_d = nc.dram_tensor("eat", [NCH * D, 4 * C], bf16, kind="ExternalInput").ap()
    ct_d = nc.dram_tensor("consts", [C, NCH], f32, kind="ExternalInput").ap()
    mk_d = nc.dram_tensor("mask", [C, C], bf16, kind="ExternalInput").ap()
    out_d = nc.dram_tensor("out", [ROWS, H], bf16, kind="ExternalOutput").ap()

    with tile.TileContext(nc) as tc:
        with (
            tc.tile_pool(name="const", bufs=1) as cpool,
            tc.tile_pool(name="bxp", bufs=9) as bxpool,
            tc.tile_pool(name="app", bufs=9) as appool,
            tc.tile_pool(name="eatp", bufs=4) as eatpool,
            tc.tile_pool(name="stp", bufs=4) as stpool,
            tc.tile_pool(name="msp", bufs=2) as mspool,
            tc.tile_pool(name="outp", bufs=4) as outpool,
            tc.tile_pool(name="ps_m", bufs=1, space="PSUM") as ps_m,
            tc.tile_pool(name="ps_out", bufs=2, space="PSUM") as ps_out,
            tc.tile_pool(name="ps_sp", bufs=1, space="PSUM") as ps_sp,
        ):
            consts_s = cpool.tile([C, NCH], f32)
            mask_s = cpool.tile([C, C], bf16)

            BX2 = {}   # (b, pair) -> [C, 2H] bf16
            AP2 = {}   # (b, pair) -> [C, 2D] bf16 (A' token-major)
            EAT = {}   # s -> [D, 4C] bf16: [Et(b0)|A't(b0)|Et(b1)|A't(b1)]
            ST = {}    # (b, s) -> [C, C] bf16
            SP = {}    # s -> [C, 256] f32 psum: s_p(b0), s_p(b1)
            OP = {}    # (b, s) -> [C, H] f32 psum
            OUT2 = {}  # (b, pair) -> [C, 2H] bf16
            MS = {}    # s -> [D, 2H] bf16: M(b0) | M(b1)

            def load_eat(s, eng=None):
                eng = eng if eng is not None else nc.sync
                EAT[s] = eatpool.tile([D, 4 * C], bf16, name=f"EAT_{s}", tag="EAT")
                eng.dma_start(out=EAT[s][:], in_=eat_d[s * D:(s + 1) * D, :])

            def load_pair(b, p, eng=None):
                eng = eng if eng is not None else nc.sync
                g = b * NCH + 2 * p
                AP2[b, p] = appool.tile([C, 2 * D], bf16, name=f"AP2_{b}_{p}", tag="AP2")
                eng.dma_start(
                    out=AP2[b, p][:].rearrange("p (two d) -> p two d", two=2),
                    in_=ap_d[g * C:(g + 2) * C, :].rearrange(
                        "(two p) d -> p two d", two=2
                    ),
                )
                BX2[b, p] = bxpool.tile([C, 2 * H], bf16, name=f"BX2_{b}_{p}", tag="BX2")
                eng.dma_start(
                    out=BX2[b, p][:].rearrange("p (two h) -> p two h", two=2),
                    in_=bx_d[g * C:(g + 2) * C, :].rearrange(
                        "(two p) h -> p two h", two=2
                    ),
                )

            def ap_view(b, s):
                off = (s % 2) * D
                return AP2[b, s // 2][:, off:off + D]

            def bx_view(b, s, lo=0, hi=H):
                off = (s % 2) * H
                return BX2[b, s // 2][:, off + lo:off + hi]

            def chain_S(s):
                SP[s] = ps_sp.tile([C, 256], f32, name=f"SP_{s}", tag="SP")
                for b in (0, 1):
                    nc.tensor.matmul(
                        out=SP[s][:, b * C:(b + 1) * C],
                        lhsT=EAT[s][0:D, (2 * b + 1) * C:(2 * b + 2) * C],
                        rhs=EAT[s][0:D, 2 * b * C:(2 * b + 1) * C],
                        start=True,
                        stop=True,
                    )

            def chain_St(s):
                for b in (0, 1):
                    ST[b, s] = stpool.tile([C, C], bf16, name=f"ST_{b}_{s}", tag="ST")
                    nc.vector.tensor_tensor(
                        out=ST[b, s][:],
                        in0=SP[s][:, b * C:(b + 1) * C],
                        in1=mask_s[:],
                        op=mult,
                    )

            # prologue: critical chunk-0 operands first, spread over two queues
            load_eat(0, nc.sync)
            nc.sync.dma_start(out=consts_s[:], in_=ct_d[:, :])
            nc.sync.dma_start(out=mask_s[:], in_=mk_d[:, :])
            load_pair(0, 0, nc.sync)
            load_eat(1, nc.scalar)
            load_pair(1, 0, nc.scalar)
            load_pair(0, 1, nc.sync)
            load_pair(1, 1, nc.scalar)

            # p-state warmup: the tensor engine only reaches 2.4GHz after ~3us
            # of continuous execution, so chew on scratch 512-col matmuls
            # (shared stationary operand -> no LDW gaps) while the first
            # operands stream in. The scratch PSUM tile comes from the ps_out
            # pool, whose first real tiles simply queue behind it on the PE.
            WARM = int(os.environ.get("KWARM", "9"))
            if WARM:
                wsrc = cpool.tile([C, 512], bf16)
                nc.gpsimd.memset(wsrc[:], 0.0)
                wdst = ps_out.tile([C, H], f32, name="wdst", tag="OP")
                for _ in range(WARM):
                    nc.tensor.matmul(
                        out=wdst[:, 0:512], lhsT=wsrc[:, 0:C], rhs=wsrc[:],
                        start=True, stop=True, skip_group_check=True,
                    )

            chain_S(0)
            chain_St(0)

            M_both = ps_m.tile([D, 2 * H], f32, name="M_both", tag="M_both")

            for s in range(NCH):
                nxt = s + 1
                # smooth prefetch on three rings: EAT on scalar, bx/A' pairs
                # on the gpsimd ring (its engine is otherwise idle), stores
                # keep the sync ring
                if nxt + 1 < NCH:
                    load_eat(nxt + 1, nc.scalar)
                pb, pp = s % 2, s // 2 + 2
                if pp < NP2:
                    load_pair(pb, pp, nc.gpsimd)

                # PE: M updates for this step (bank-aligned per-batch splits).
                # start=True arms the WHOLE 2KB psum zero-region: b1's (0,256)
                # shares a bank with b0's (512,768), so it must NOT re-arm it
                # (its bytes are already pending from b0's start, making its
                # first write an overwrite as required).
                if s < NCH - 1:
                    MS[nxt] = mspool.tile([D, 2 * H], bf16, name=f"MS_{nxt}", tag="MS")
                    for b in (0, 1):
                        for lo, hi in MUP_SPLIT[b]:
                            nc.tensor.matmul(
                                out=M_both[:, b * H + lo:b * H + hi],
                                lhsT=ap_view(b, s),
                                rhs=bx_view(b, s, lo, hi),
                                start=(s == 0 and not (b == 1 and lo == 0)),
                                stop=True,
                                skip_group_check=True,
                            )
                    nc.scalar.copy(out=MS[nxt][:], in_=M_both[:])

                # PE: score matmuls for next step
                if nxt < NCH:
                    chain_S(nxt)
                    chain_St(nxt)

                # PE: output accumulation + final AXPY per batch
                for b in (0, 1):
                    OP[b, s] = ps_out.tile([C, H], f32, name=f"OP_{b}_{s}", tag="OP")
                    if s > 0:
                        for lo, hi in ((0, 512), (512, H)):
                            nc.tensor.matmul(
                                out=OP[b, s][:, lo:hi],
                                lhsT=EAT[s][0:D, 2 * b * C:(2 * b + 1) * C],
                                rhs=MS[s][0:D, b * H + lo:b * H + hi],
                                start=True,
                                stop=False,
                            )
                    for lo, hi in ((0, 512), (512, H)):
                        nc.tensor.matmul(
                            out=OP[b, s][:, lo:hi],
                            lhsT=ST[b, s][:],
                            rhs=bx_view(b, s, lo, hi),
                            start=(s == 0),
                            stop=True,
                        )
                    # res = OP * (1/(j+1)) + bx -> bf16 (DVE)
                    if s % 2 == 0:
                        OUT2[b, s // 2] = outpool.tile(
                            [C, 2 * H], bf16, name=f"OUT2_{b}_{s // 2}", tag="OUT2"
                        )
                    ov = OUT2[b, s // 2][:, (s % 2) * H:(s % 2 + 1) * H]
                    nc.vector.scalar_tensor_tensor(
                        out=ov,
                        in0=OP[b, s][:],
                        scalar=consts_s[:, s:s + 1],
                        in1=bx_view(b, s),
                        op0=mult,
                        op1=add,
                    )

                # out DMA per completed pair
                if s % 2 == 1:
                    for b in (0, 1):
                        g = b * NCH + s
                        nc.sync.dma_start(
                            out=out_d[(g - 1) * C:(g + 1) * C, :].rearrange(
                                "(two p) h -> p two h", two=2
                            ),
                            in_=OUT2[b, s // 2][:].rearrange(
                                "p (two h) -> p two h", two=2
                            ),
                        )

    # Adjacent PE matmuls sharing a stationary operand reload it redundantly;
    # mark the second of each such pair as pre-loaded.
    for blk in nc.m.functions[0].blocks:
        last = None
        for inst in blk.instructions:
            if getattr(inst, "engine", None) != mybir.EngineType.PE:
                continue
            if not isinstance(inst, mybir.InstMatmult):
                if isinstance(inst, (mybir.InstLdweights,)):
                    last = None
                continue
            if (
                last is not None
                and not inst.is_transpose
                and not last.is_transpose
                and inst.ins[1].memref == last.ins[1].memref
                and inst.ins[1].offset == last.ins[1].offset
                and inst.ins[1].ap == last.ins[1].ap
            ):
                inst.ldweights = True
            last = inst

    nc.compile()
    _compiled[key] = nc
    return nc


def _np_consts():
    j = np.arange(L, dtype=np.float64)
    inv = (1.0 / (j + 1.0)).astype(np.float32).reshape(NCH, C).T
    consts = np.ascontiguousarray(inv)  # [C, NCH], col c = 1/(c*128+i+1)
    mask01 = np.triu(np.ones((C, C), np.float32), 1)
    return consts, mask01


def _in_maps(bert_x, x, ae, w):
    import ml_dtypes

    bert_x = np.asarray(bert_x, dtype=np.float32)
    x = np.asarray(x)
    ae = np.asarray(ae, dtype=np.float32)
    w = np.asarray(w, dtype=np.float32)

    eaw = np.concatenate([ae, ae @ w], axis=1)          # [V, 2D] f32
    EA = eaw[x]                                         # [B, L, 2D] f32
    scale_i = (np.arange(L, dtype=np.float64) + 1.0).astype(np.float32)
    EA[:, :, D:] *= scale_i[None, :, None]
    EAb = EA.astype(ml_dtypes.bfloat16)                 # [B, L, 2D]
    bxb = np.ascontiguousarray(bert_x.astype(ml_dtypes.bfloat16))

    # d-major per-chunk stationary blocks, same bf16 values as EAb:
    # eat[core, s*D:(s+1)*D, :] = [Et(b0) | A't(b0) | Et(b1) | A't(b1)]
    EAc = EAb.reshape(NCORES, BPC, NCH, C, 2 * D)
    # -> [cores, NCH, D, b*2+half blocks of C]
    blocks = np.transpose(EAc, (0, 2, 1, 4, 3))         # [cores,NCH,BPC,2D,C]
    blocks = blocks.reshape(NCORES, NCH, BPC * 2, D, C)
    eat = np.transpose(blocks, (0, 1, 3, 2, 4)).reshape(NCORES, NCH * D, 4 * C)
    eat = np.ascontiguousarray(eat)

    consts, mask01 = _np_consts()
    mask_b = np.ascontiguousarray(mask01.astype(ml_dtypes.bfloat16))

    maps = []
    for k in range(NCORES):
        maps.append(
            {
                "bx": bxb[k * BPC:(k + 1) * BPC].reshape(ROWS, H),
                "apm": np.ascontiguousarray(
                    EAb[k * BPC:(k + 1) * BPC, :, D:].reshape(ROWS, D)
                ),
                "eat": eat[k],
                "consts": consts,
                "mask": mask_b,
            }
        )
    return maps


def _run(bert_x, x, ae, w, trace=False):
    from concourse import bass_utils

    nc = _build()
    maps = _in_maps(bert_x, x, ae, w)
    res = bass_utils.run_bass_kernel_spmd(
        nc, maps, core_ids=list(range(NCORES)), trace=trace
    )
    out = np.concatenate(
        [
            res.results[k]["out"].astype(np.float32).reshape(BPC, L, H)
            for k in range(NCORES)
        ],
        axis=0,
    )
    return out, res


def kernel(bert_x, x, ae, w):
    out, _ = _run(bert_x, x, ae, w, trace=False)
    return out


# revision 46
# speedup vs baseline: 1.1745x; 1.1745x over previous
"""Trainium2 Bass kernel for nn_CausalLayer (bilinear causal mixing layer).

Math (per batch b):
    E = ae[x]                                # [L, D] gather
    S[i,j] = E_i @ w @ E_j                   # bilinear pairwise score
    coef[i,j] = (i+1)/(j+1) for i<j else 0
    res[:,j] = bx[:,j] + sum_i coef[i,j]*S[i,j]*bx[:,i]

Chunked linear-attention identity, per 128-token chunk c with
a'_i = (i+1) * (w^T e_i):
    res_j = bx_j + (1/(j+1)) * [ E_j @ M_c + sum_{i<j in c} (a'_i . e_j) bx_i ]
    M_c   = sum_{i in chunks < c} a'_i bx_i^T      ([D, H] running state)

Host prep: the fused gather table row [ae[v] | (ae@w)[v]] is gathered and
position-scaled on host, and handed to the device in the two layouts the PE
consumes: A' token-major ([ROWS, D], the Mup stationary operand) and
[Et | A't] d-major per chunk ([NCH*D, 4C], S/EM stationary operands, both
batches side by side). This removes all on-device transposes/copies for the
score path at the cost of ~1MB/core of extra DMA.

Device: the two local batches are interleaved per chunk-step. All PE
operands sit at SBUF partition base 0 (mixed-base row-tiled matmul pairs
hang TRN2); per-batch data is side-by-side on the free axis, with the M
state as one [64, 2H] PSUM accumulator (bank-aligned splits). Wire dtypes
bf16 (incl. the output, upcast on host); f32 accumulation in PSUM; the mask
is a constant 0/1 strictly-upper [128,128] tile.

Sharding: batch-parallel, 2 of 16 batches per core, no cross-core comms.
"""

import os
import sys

for _p in ("/opt/trn_rl_repo", "/root/.axon_site/_ro/trn_rl_repo"):
    if os.path.isdir(_p) and _p not in sys.path:
        sys.path.insert(0, _p)

import numpy as np

B, L, H = 16, 2048, 768
V, D = 30000, 64
NCORES = 8
BPC = B // NCORES          # batches per core
C = 128                    # chunk (tile) size along sequence
NCH = L // C               # chunks per batch
ROWS = BPC * L             # rows per core
NP2 = NCH // 2             # chunk pairs per batch

_compiled = {}

# PSUM-bank-aligned column splits for the [64, 2H] M accumulator
MUP_SPLIT = (((0, 512), (512, 768)), ((0, 256), (256, 768)))


def _build():
    key = ("v6", os.environ.get("KWARM", "9"))
    if key in _compiled:
        return _compiled[key]

    import concourse.bacc as bacc
    import concourse.bass as bass
    import concourse.mybir as mybir
    import concourse.tile as tile

    f32 = mybir.dt.float32
    bf16 = mybir.dt.bfloat16
    mult = mybir.AluOpType.mult
    add = mybir.AluOpType.add

    nc = bacc.Bacc(
        "TRN2",
        target_bir_lowering=False,
        debug=False,
        enable_asserts=False,
        num_devices=NCORES,
    )

    bx_d = nc.dram_tensor("bx", [ROWS, H], bf16, kind="ExternalInput").ap()
    ap_d = nc.dram_tensor("apm", [ROWS, D], bf16, kind="ExternalInput").ap()
    eat_d = nc.dram_tensor("eat", [NCH * D, 4 * C], bf16, kind="ExternalInput").ap()
    ct_d = nc.dram_tensor("consts", [C, NCH], f32, kind="ExternalInput").ap()
    mk_d = nc.dram_tensor("mask", [C, C], bf16, kind="ExternalInput").ap()
    out_d = nc.dram_tensor("out", [ROWS, H], bf16, kind="ExternalOutput").ap()

    with tile.TileContext(nc) as tc:
        with (
            tc.tile_pool(name="const", bufs=1) as cpool,
            tc.tile_pool(name="bxp", bufs=9) as bxpool,
            tc.tile_pool(name="app", bufs=9) as appool,
            tc.tile_pool(name="eatp", bufs=4) as eatpool,
            tc.tile_pool(name="stp", bufs=4) as stpool,
            tc.tile_pool(name="msp", bufs=2) as mspool,
            tc.tile_pool(name="outp", bufs=4) as outpool,
            tc.tile_pool(name="ps_m", bufs=1, space="PSUM") as ps_m,
            tc.tile_pool(name="ps_out", bufs=2, space="PSUM") as ps_out,
            tc.tile_pool(name="ps_sp", bufs=1, space="PSUM") as ps_sp,
        ):
            consts_s = cpool.tile([C, NCH], f32)
            mask_s = cpool.tile([C, C], bf16)

            BX2 = {}   # (b, pair) -> [C, 2H] bf16
            AP2 = {}   # (b, pair) -> [C, 2D] bf16 (A' token-major)
            EAT = {}   # s -> [D, 4C] bf16: [Et(b0)|A't(b0)|Et(b1)|A't(b1)]
            ST = {}    # (b, s) -> [C, C] bf16
            SP = {}    # s -> [C, 256] f32 psum: s_p(b0), s_p(b1)
            OP = {}    # (b, s) -> [C, H] f32 psum
            OUT2 = {}  # (b, pair) -> [C, 2H] bf16
            MS = {}    # s -> [D, 2H] bf16: M(b0) | M(b1)

            def load_eat(s, eng=None):
                eng = eng if eng is not None else nc.sync
                EAT[s] = eatpool.tile([D, 4 * C], bf16, name=f"EAT_{s}", tag="EAT")
                eng.dma_start(out=EAT[s][:], in_=eat_d[s * D:(s + 1) * D, :])

            def load_pair(b, p, eng=None):
                eng = eng if eng is not None else nc.sync
                g = b * NCH + 2 * p
                AP2[b, p] = appool.tile([C, 2 * D], bf16, name=f"AP2_{b}_{p}", tag="AP2")
                eng.dma_start(
                    out=AP2[b, p][:].rearrange("p (two d) -> p two d", two=2),
                    in_=ap_d[g * C:(g + 2) * C, :].rearrange(
                        "(two p) d -> p two d", two=2
                    ),
                )
                BX2[b, p] = bxpool.tile([C, 2 * H], bf16, name=f"BX2_{b}_{p}", tag="BX2")
                eng.dma_start(
                    out=BX2[b, p][:].rearrange("p (two h) -> p two h", two=2),
                    in_=bx_d[g * C:(g + 2) * C, :].rearrange(
                        "(two p) h -> p two h", two=2
                    ),
                )

            def ap_view(b, s):
                off = (s % 2) * D
                return AP2[b, s // 2][:, off:off + D]

            def bx_view(b, s, lo=0, hi=H):
                off = (s % 2) * H
                return BX2[b, s // 2][:, off + lo:off + hi]

            def chain_S(s):
                SP[s] = ps_sp.tile([C, 256], f32, name=f"SP_{s}", tag="SP")
                for b in (0, 1):
                    nc.tensor.matmul(
                        out=SP[s][:, b * C:(b + 1) * C],
                        lhsT=EAT[s][0:D, (2 * b + 1) * C:(2 * b + 2) * C],
                        rhs=EAT[s][0:D, 2 * b * C:(2 * b + 1) * C],
                        start=True,
                        stop=True,
                    )

            def chain_St(s):
                for b in (0, 1):
                    ST[b, s] = stpool.tile([C, C], bf16, name=f"ST_{b}_{s}", tag="ST")
                    nc.vector.tensor_tensor(
                        out=ST[b, s][:],
                        in0=SP[s][:, b * C:(b + 1) * C],
                        in1=mask_s[:],
                        op=mult,
                    )

            # prologue: critical chunk-0 operands first, spread over two queues
            load_eat(0, nc.sync)
            nc.sync.dma_start(out=consts_s[:], in_=ct_d[:, :])
            nc.sync.dma_start(out=mask_s[:], in_=mk_d[:, :])
            load_pair(0, 0, nc.sync)
            load_eat(1, nc.scalar)
            load_pair(1, 0, nc.scalar)
            load_pair(0, 1, nc.gpsimd)
            load_pair(1, 1, nc.gpsimd)

            # p-state warmup: the tensor engine only reaches 2.4GHz after ~3us
            # of continuous execution, so chew on scratch 512-col matmuls
            # (shared stationary operand -> no LDW gaps) while the first
            # operands stream in. The scratch PSUM tile comes from the ps_out
            # pool, whose first real tiles simply queue behind it on the PE.
            WARM = int(os.environ.get("KWARM", "9"))
            if WARM:
                wsrc = cpool.tile([C, 512], bf16)
                nc.gpsimd.memset(wsrc[:], 0.0)
                wdst = ps_out.tile([C, H], f32, name="wdst", tag="OP")
                for _ in range(WARM):
                    nc.tensor.matmul(
                        out=wdst[:, 0:512], lhsT=wsrc[:, 0:C], rhs=wsrc[:],
                        start=True, stop=True, skip_group_check=True,
                    )

            chain_S(0)
            chain_St(0)

            M_both = ps_m.tile([D, 2 * H], f32, name="M_both", tag="M_both")

            for s in range(NCH):
                nxt = s + 1
                # smooth prefetch on three rings: EAT on scalar, bx/A' pairs
                # on the gpsimd ring (its engine is otherwise idle), stores
                # keep the sync ring
                if nxt + 1 < NCH:
                    load_eat(nxt + 1, nc.scalar)
                pb, pp = s % 2, s // 2 + 2
                if pp < NP2:
                    load_pair(pb, pp, nc.gpsimd)

                # PE: M updates for this step (bank-aligned per-batch splits).
                # start=True arms the WHOLE 2KB psum zero-region: b1's (0,256)
                # shares a bank with b0's (512,768), so it must NOT re-arm it
                # (its bytes are already pending from b0's start, making its
                # first write an overwrite as required).
                if s < NCH - 1:
                    MS[nxt] = mspool.tile([D, 2 * H], bf16, name=f"MS_{nxt}", tag="MS")
                    for b in (0, 1):
                        for lo, hi in MUP_SPLIT[b]:
                            nc.tensor.matmul(
                                out=M_both[:, b * H + lo:b * H + hi],
                                lhsT=ap_view(b, s),
                                rhs=bx_view(b, s, lo, hi),
                                start=(s == 0 and not (b == 1 and lo == 0)),
                                stop=True,
                                skip_group_check=True,
                            )
                    nc.scalar.copy(out=MS[nxt][:], in_=M_both[:])

                # PE: score matmuls for next step
                if nxt < NCH:
                    chain_S(nxt)
                    chain_St(nxt)

                # PE: output accumulation + final AXPY per batch
                for b in (0, 1):
                    OP[b, s] = ps_out.tile([C, H], f32, name=f"OP_{b}_{s}", tag="OP")
                    if s > 0:
                        for lo, hi in ((0, 512), (512, H)):
                            nc.tensor.matmul(
                                out=OP[b, s][:, lo:hi],
                                lhsT=EAT[s][0:D, 2 * b * C:(2 * b + 1) * C],
                                rhs=MS[s][0:D, b * H + lo:b * H + hi],
                                start=True,
                                stop=False,
                            )
                    for lo, hi in ((0, 512), (512, H)):
                        nc.tensor.matmul(
                            out=OP[b, s][:, lo:hi],
                            lhsT=ST[b, s][:],
                            rhs=bx_view(b, s, lo, hi),
                            start=(s == 0),
                            stop=True,
                        )
                    # res = OP * (1/(j+1)) + bx -> bf16 (DVE)
                    if s % 2 == 0:
                        OUT2[b, s // 2] = outpool.tile(
                            [C, 2 * H], bf16, name=f"OUT2_{b}_{s // 2}", tag="OUT2"
                        )
                    ov = OUT2[b, s // 2][:, (s % 2) * H:(s % 2 + 1) * H]
                    nc.vector.scalar_tensor_tensor(
                        out=ov,
                        in0=OP[b, s][:],
                        scalar=consts_s[:, s:s + 1],
                        in1=bx_view(b, s),
                        op0=mult,
                        op1=add,
                    )

                # out DMA per completed pair
                if s % 2 == 1:
                    for b in (0, 1):
                        g = b * NCH + s
                        nc.sync.dma_start(
                            out=out_d[(g - 1) * C:(g + 1) * C, :].rearrange(
                                "(two p) h -> p two h", two=2
                            ),
                            in_=OUT2[b, s // 2][:].rearrange(
                                "p (two h) -> p two h", two=2
                            ),
                        )

    # Adjacent PE matmuls sharing a stationary operand reload it redundantly;
    # mark the second of each such pair as pre-loaded.
    for blk in nc.m.functions[0].blocks:
        last = None
        for inst in blk.instructions:
            if getattr(inst, "engine", None) != mybir.EngineType.PE:
                continue
            if not isinstance(inst, mybir.InstMatmult):
                if isinstance(inst, (mybir.InstLdweights,)):
                    last = None
                continue
            if (
                last is not None
                and not inst.is_transpose
                and not last.is_transpose
                and inst.ins[1].memref == last.ins[1].memref
                and inst.ins[1].offset == last.ins[1].offset
                and inst.ins[1].ap == last.ins[1].ap
            ):
                inst.ldweights = True
            last = inst

    nc.compile()
    _compiled[key] = nc
    return nc


def _np_consts():
    j = np.arange(L, dtype=np.float64)
    inv = (1.0 / (j + 1.0)).astype(np.float32).reshape(NCH, C).T
    consts = np.ascontiguousarray(inv)  # [C, NCH], col c = 1/(c*128+i+1)
    mask01 = np.triu(np.ones((C, C), np.float32), 1)
    return consts, mask01


def _in_maps(bert_x, x, ae, w):
    import ml_dtypes

    bert_x = np.asarray(bert_x, dtype=np.float32)
    x = np.asarray(x)
    ae = np.asarray(ae, dtype=np.float32)
    w = np.asarray(w, dtype=np.float32)

    eaw = np.concatenate([ae, ae @ w], axis=1)          # [V, 2D] f32
    EA = eaw[x]                                         # [B, L, 2D] f32
    scale_i = (np.arange(L, dtype=np.float64) + 1.0).astype(np.float32)
    EA[:, :, D:] *= scale_i[None, :, None]
    EAb = EA.astype(ml_dtypes.bfloat16)                 # [B, L, 2D]
    bxb = np.ascontiguousarray(bert_x.astype(ml_dtypes.bfloat16))

    # d-major per-chunk stationary blocks, same bf16 values as EAb:
    # eat[core, s*D:(s+1)*D, :] = [Et(b0) | A't(b0) | Et(b1) | A't(b1)]
    EAc = EAb.reshape(NCORES, BPC, NCH, C, 2 * D)
    # -> [cores, NCH, D, b*2+half blocks of C]
    blocks = np.transpose(EAc, (0, 2, 1, 4, 3))         # [cores,NCH,BPC,2D,C]
    blocks = blocks.reshape(NCORES, NCH, BPC * 2, D, C)
    eat = np.transpose(blocks, (0, 1, 3, 2, 4)).reshape(NCORES, NCH * D, 4 * C)
    eat = np.ascontiguousarray(eat)

    consts, mask01 = _np_consts()
    mask_b = np.ascontiguousarray(mask01.astype(ml_dtypes.bfloat16))

    maps = []
    for k in range(NCORES):
        maps.append(
            {
                "bx": bxb[k * BPC:(k + 1) * BPC].reshape(ROWS, H),
                "apm": np.ascontiguousarray(
                    EAb[k * BPC:(k + 1) * BPC, :, D:].reshape(ROWS, D)
                ),
                "eat": eat[k],
                "consts": consts,
                "mask": mask_b,
            }
        )
    return maps


def _run(bert_x, x, ae, w, trace=False):
    from concourse import bass_utils

    nc = _build()
    maps = _in_maps(bert_x, x, ae, w)
    res = bass_utils.run_bass_kernel_spmd(
        nc, maps, core_ids=list(range(NCORES)), trace=trace
    )
    out = np.concatenate(
        [
            res.results[k]["out"].astype(np.float32).reshape(BPC, L, H)
            for k in range(NCORES)
        ],
        axis=0,
    )
    return out, res


def kernel(bert_x, x, ae, w):
    out, _ = _run(bert_x, x, ae, w, trace=False)
    return out
